# revision 42
# baseline (speedup 1.0000x reference)
"""GCNGraphDTA Trainium2 kernel (v3).

Graphs are independent 25-node blocks, so each GCN layer
    h' = relu( D^-1/2 (A+I) D^-1/2 (h W) + b )
is dense linear algebra with a block-diagonal normalized adjacency.
Host prep builds, per graph, the 25x25 matrix
    AT_g[u, v] = dinv[u] * dinv[v] * count(u->v) + dinv[u]^2 * delta_uv
(the transpose of the propagation matrix) in fp8e4m3, packs 5 graphs
into a 125x125 block-diagonal tile, and hands each of the 8 cores its
256 graphs (padded to 260 = 52 tiles) plus replicated weights.

Per layer, per core: 52 hW matmuls (lhsT = feat-major H group, rhs = W)
-> PSUM -> fp16 cast -> 52 agg matmuls (lhsT = node-major hW, rhs = fp8
AT tile) -> PSUM -> relu(+bias) -> feat-major H of the next layer.

Measured engine facts this schedule is built around (v1/v2 traces):
  - PSUM->SBUF moves run ONLY on ACT (~0.74 cols/ns at 512) and DVE
    (~0.69); gpsimd has no PSUM access and no tensor_tensor/X-reduce.
    The per-instruction overhead (~230ns) argues for 1024-col relus
    (agg batches of 8 groups, 2 PSUM banks) and 512-col hw casts.
  - The HAM clock gate drops the PE to 4/8 after ~1us of PE idle and
    takes >10us to recover: the input DMA is therefore split so the PE
    never waits: W1+biases ride the scalar queue (lands ~1us), fp8 xT
    + AT chunks 0/2/3 on sync, big fp16 consts + AT chunk 1 on gpsimd
    (~85 GB/s per queue), each arriving just before its consumer.
  - fp8e4m3 for AT and xT: 2.7e-3 end-to-end rel err vs the 2e-2 gate
    (MLP weights in fp8 would cost 4e-2, so they stay fp16).
  - Global max pool = pairwise max tree on DVE over fp16 SBUF (2x DVE
    mode; reduce_max gets no fast mode), 3 chunks interleaved into
    layer 3; MLP chunked in 2 right behind them so the tail is short.
"""

import numpy as np
import ml_dtypes

import concourse.bass as bass
import concourse.mybir as mybir
import concourse.tile as tile
from concourse.bass_utils import run_bass_kernel_spmd

N_CORES = 8
N_GRAPHS = 2048
NPG = 25               # nodes per graph
N_NODES = N_GRAPHS * NPG
F_IN = 13
HID = 128
PROT = 128
GPC = N_GRAPHS // N_CORES      # 256 graphs per core
PAD_G = 260                    # padded to a multiple of 5
GPG = 5                        # graphs per 125-row group
GROUPS = PAD_G // GPG          # 52
GW = GPG * NPG                 # 125 = group width (nodes)
GS = 128                       # group column stride in H layout (PSUM align)
COLS_A = GROUPS * GW           # 6500: AT columns (dense 125-wide groups)
COLS_H = GROUPS * GS           # 6656: H/xT columns (128-wide groups)
BATCH = 4                      # hW groups per batch -> 512 cols = 1 bank
N_BATCH = GROUPS // BATCH      # 13
ABATCH = 2 * BATCH             # agg groups per batch -> 1024 cols = 2 banks
N_ABATCH = (GROUPS + ABATCH - 1) // ABATCH   # 7 (last has 4 groups)
N_WARM = 12                    # dummy matmuls to warm the PE clock gate
WARM_N = 512                   # cols per dummy matmul

# packed fp16 const blob column offsets; cols 0:CW_EARLY are needed from
# layer 2 on and ride an early DMA, the MLP part (Wf1, PT) arrives late
CW_W2 = 0
CW_W3 = HID
CW_WF2 = 2 * HID               # wf2a col, wf2b col
CW_EARLY = CW_WF2 + 2          # 258
CW_WF1 = CW_EARLY              # Wf1 rows 0:128 at +0, rows 128:256 at +256
CW_PT = CW_WF1 + 512
CW_COLS = CW_PT + GPC          # 1026

# pool tree rest-chunks in groups: fired after the agg batch that
# completes them
POOL_CH = [(0, 26), (26, 52)]
# mlp chunks in graphs
MLP_CH = [(0, 130), (130, GPC)]

F32 = mybir.dt.float32
F16 = mybir.dt.float16
F8 = mybir.dt.float8e4


# mover schedule: greedy finish-time balance between DVE(0) / ACT(1)
# using measured per-instruction times
def _mover_cost(eng, cols):
    per = {0: 1.04, 1: 0.833}[eng]
    return cols * per + 230.0


def _mover_schedule(n_cast, cast_cols, n_relu, relu_cols, dve_preload):
    load = [dve_preload, 0.0]
    out = []
    for kind, cols in [("c", cast_cols)] * n_cast + [("r", relu_cols)] * n_relu:
        e = min(range(2), key=lambda i: load[i] + _mover_cost(i, cols))
        load[e] += _mover_cost(e, cols)
        out.append(e)
    return out[:n_cast], out[n_cast:]


def _split_multi_waits(nc):
    """This container's walrus build accepts at most one sem wait per
    instruction (two for EventSemaphore). Tile emits multi-waits freely, so
    hoist the extras onto same-engine NoOps inserted just before."""
    for f in nc.m.functions:
        for blk in f.blocks:
            new_insts = []
            for inst in blk.instructions:
                si = getattr(inst, "sync_info", None)
                cap = 2 if inst.opcode == "EventSemaphore" else 1
                if si is not None and si.on_wait and len(si.on_wait) > cap:
                    waits = list(si.on_wait)
                    for i, w in enumerate(waits[:-cap]):
                        new_insts.append(mybir.InstNoOp(
                            name=f"{inst.name}-ws{i}",
                            engine=inst.engine,
                            bass_nofuse=True,
                            sync_info=mybir.SyncInfo(on_wait=[w], on_update=[]),
                        ))
                    si.on_wait = waits[-cap:]
                new_insts.append(inst)
            blk.instructions[:] = new_insts


def _build_program():
    nc = bass.Bass()

    xT = nc.dram_tensor("xT", [F_IN, COLS_H], F16, kind="ExternalInput")
    AT = nc.dram_tensor("AT", [GW, COLS_A], F8, kind="ExternalInput")
    W1B = nc.dram_tensor("W1B", [F_IN, HID], F16, kind="ExternalInput")
    CW = nc.dram_tensor("CW", [HID, CW_COLS], F16, kind="ExternalInput")
    CB = nc.dram_tensor("CB", [HID, 6], F32, kind="ExternalInput")
    OUT = nc.dram_tensor("out", [1, GPC], F32, kind="ExternalOutput")

    relu = mybir.ActivationFunctionType.Relu
    ident = mybir.ActivationFunctionType.Identity

    with tile.TileContext(nc) as tc:
        with (
            tc.tile_pool(name="const", bufs=1) as cpool,
            tc.tile_pool(name="hw", bufs=16) as hwpool,
            tc.tile_pool(name="tmp", bufs=2) as tmppool,
            tc.tile_pool(name="mm", bufs=4, space="PSUM") as mmpool,
            tc.tile_pool(name="agg", bufs=2, space="PSUM") as aggpool,
        ):
            # ---- persistent SBUF tensors ----
            cw_sb = cpool.tile([HID, CW_COLS], F16)
            w1_sb = cpool.tile([F_IN, HID], F16)
            cb_sb = cpool.tile([HID, 6], F32)
            xT_sb = cpool.tile([F_IN, COLS_H], F16)
            at_sb = cpool.tile([GW, COLS_A], F8)
            h1_sb = cpool.tile([HID, COLS_H], F16)
            h2_sb = cpool.tile([HID, COLS_H], F16)
            h3_sb = cpool.tile([HID, COLS_H], F16)
            drug_sb = cpool.tile([HID, PAD_G], F16)
            fc1a_sb = cpool.tile([HID, GPC], F16)
            fc1b_sb = cpool.tile([HID, GPC], F16)
            out_sb = cpool.tile([1, GPC], F32)
            warm_sb = cpool.tile([HID, HID + WARM_N], F16)

            # ---- input DMAs first: transfers cannot start before ~9us
            # (fixed engine-barrier preamble + DGE latency), each dma_start
            # costs ~0.6us of its queue's sequencer, and a queue sustains
            # ~110 GB/s -- so: few big transfers, in consumption order,
            # with layer 1's critical path (W1, first xT cols) leading ----
            nc.vector.memset(warm_sb[:], 0.0)
            nc.sync.dma_start(out=w1_sb[:], in_=W1B[:])
            nc.sync.dma_start(out=xT_sb[:], in_=xT[:])
            nc.scalar.dma_start(out=cb_sb[:], in_=CB[:])
            acut = [0, 13 * GW, 26 * GW, 39 * GW, COLS_A]
            nc.sync.dma_start(out=at_sb[:, acut[0]:acut[1]],
                              in_=AT[:, acut[0]:acut[1]])
            nc.gpsimd.dma_start(out=cw_sb[:, 0:CW_EARLY],
                                in_=CW[:, 0:CW_EARLY])
            nc.gpsimd.dma_start(out=at_sb[:, acut[1]:acut[2]],
                                in_=AT[:, acut[1]:acut[2]])
            nc.sync.dma_start(out=at_sb[:, acut[2]:acut[3]],
                              in_=AT[:, acut[2]:acut[3]])
            nc.gpsimd.dma_start(out=at_sb[:, acut[3]:acut[4]],
                                in_=AT[:, acut[3]:acut[4]])
            nc.gpsimd.dma_start(out=cw_sb[:, CW_EARLY:CW_COLS],
                                in_=CW[:, CW_EARLY:CW_COLS])

            # ---- PE warm-up: dummy matmuls during the DMA head keep the
            # HAM clock gate ramping until the real stream begins ----
            for _ in range(N_WARM):
                warm_ps = mmpool.tile([HID, WARM_N], F32, tag="mm",
                                      name="warm_ps")
                nc.tensor.matmul(out=warm_ps[:], lhsT=warm_sb[:, 0:HID],
                                 rhs=warm_sb[:, HID:HID + WARM_N],
                                 start=True, stop=True)

            # ---- mover helpers ----
            def emit_cast(eng, out, in_):
                if eng == 0:
                    nc.vector.tensor_copy(out=out, in_=in_)
                else:
                    nc.scalar.copy(out=out, in_=in_)

            def emit_relu(eng, out, in_, b_sb):
                if eng == 1:
                    nc.scalar.activation(out=out, in_=in_, func=relu,
                                         bias=b_sb)
                else:
                    nc.vector.tensor_scalar(out=out, in0=in_,
                                            scalar1=b_sb, scalar2=0.0,
                                            op0=mybir.AluOpType.add,
                                            op1=mybir.AluOpType.max)

            # ---- global max pool: pairwise max tree on DVE over fp16 SBUF
            # h3 (the verifier allows only one PSUM input per instruction,
            # so the tree cannot read agg PSUM directly) ----
            def emit_pool(c):
                g0, g1 = POOL_CH[c]
                ng = g1 - g0
                v = (h3_sb[:, g0 * GS:g1 * GS]
                     .rearrange("p (g c2) -> p g c2", c2=GS)[:, :, 0:GW]
                     .rearrange("p g (j n) -> p g j n", n=NPG))
                ta = tmppool.tile([HID, ng * GPG * 13], F16, name="pool_a")
                a = ta.rearrange("p (g j n) -> p g j n", j=GPG, n=13)
                nc.vector.tensor_tensor(
                    out=a, in0=v[:, :, :, 0:13], in1=v[:, :, :, 12:25],
                    op=mybir.AluOpType.max)
                tb = tmppool.tile([HID, ng * GPG * 7], F16, name="pool_b")
                b = tb.rearrange("p (g j n) -> p g j n", j=GPG, n=7)
                nc.vector.tensor_tensor(
                    out=b, in0=a[:, :, :, 0:7], in1=a[:, :, :, 6:13],
                    op=mybir.AluOpType.max)
                nc.vector.tensor_tensor(
                    out=a[:, :, :, 0:4], in0=b[:, :, :, 0:4],
                    in1=b[:, :, :, 3:7], op=mybir.AluOpType.max)
                nc.vector.tensor_tensor(
                    out=b[:, :, :, 0:2], in0=a[:, :, :, 0:2],
                    in1=a[:, :, :, 2:4], op=mybir.AluOpType.max)
                dv = (drug_sb[:, g0 * GPG:g1 * GPG]
                      .rearrange("p (g j n) -> p g j n", j=GPG, n=1))
                nc.vector.tensor_tensor(
                    out=dv, in0=b[:, :, :, 0:1], in1=b[:, :, :, 1:2],
                    op=mybir.AluOpType.max)

            # ---- MLP chunk: relu([drug; prot] @ Wf1 + bf1) @ Wf2 + bf2 ----
            def emit_mlp(c):
                g0, g1 = MLP_CH[c]
                n = g1 - g0
                gs = slice(g0, g1)
                fc2_ps = aggpool.tile([1, n], F32, tag="agg", name="fc2_ps")
                for mc, fc1_sb in enumerate([fc1a_sb, fc1b_sb]):
                    fc1_ps = mmpool.tile([HID, n], F32, tag="mm",
                                         name="fc1_ps")
                    ms = slice(CW_WF1 + mc * HID, CW_WF1 + (mc + 1) * HID)
                    nc.tensor.matmul(out=fc1_ps[:], lhsT=cw_sb[:, ms],
                                     rhs=drug_sb[:, gs],
                                     start=True, stop=False)
                    ms2 = slice(CW_WF1 + 256 + mc * HID,
                                CW_WF1 + 256 + (mc + 1) * HID)
                    nc.tensor.matmul(out=fc1_ps[:], lhsT=cw_sb[:, ms2],
                                     rhs=cw_sb[:, CW_PT + g0:CW_PT + g1],
                                     start=False, stop=True)
                    nc.scalar.activation(out=fc1_sb[:, gs], in_=fc1_ps[:],
                                         func=relu,
                                         bias=cb_sb[:, 3 + mc:4 + mc])
                nc.tensor.matmul(out=fc2_ps[:],
                                 lhsT=cw_sb[:, CW_WF2:CW_WF2 + 1],
                                 rhs=fc1a_sb[:, gs], start=True, stop=False)
                nc.tensor.matmul(out=fc2_ps[:],
                                 lhsT=cw_sb[:, CW_WF2 + 1:CW_WF2 + 2],
                                 rhs=fc1b_sb[:, gs], start=False, stop=True)
                nc.scalar.activation(out=out_sb[:, gs], in_=fc2_ps[:],
                                     func=ident, bias=cb_sb[0:1, 5:6])

            # ---- 3 GCN layers ----
            layers = [
                (xT_sb, F_IN, w1_sb, 0, h1_sb),
                (h1_sb, HID, None, 1, h2_sb),
                (h2_sb, HID, None, 2, h3_sb),
            ]

            for li, (h_in, kdim, w_ap, wi, h_out) in enumerate(layers):
                if w_ap is None:
                    off = CW_W2 if li == 1 else CW_W3
                    w_ap = cw_sb[0:kdim, off:off + HID]
                b_ap = cb_sb[:, wi:wi + 1]
                cast_s, relu_s = _mover_schedule(
                    N_BATCH, BATCH * GS, N_ABATCH, ABATCH * GS,
                    3200.0 if li == 2 else 0.0)

                def emit_agg(B, hw_sbs):
                    g0 = B * ABATCH
                    g1 = min(GROUPS, g0 + ABATCH)
                    agg_ps = aggpool.tile([HID, (g1 - g0) * GS], F32,
                                          tag="agg", name="agg_ps")
                    for gi, g in enumerate(range(g0, g1)):
                        hw_sb = hw_sbs[gi // BATCH]
                        hi = gi % BATCH
                        nc.tensor.matmul(
                            out=agg_ps[:, gi * GS:gi * GS + GW],
                            lhsT=hw_sb[0:GW, hi * HID:(hi + 1) * HID],
                            rhs=at_sb[:, g * GW:(g + 1) * GW],
                            start=True, stop=True,
                        )
                    h_slice = h_out[:, g0 * GS:g1 * GS]
                    emit_relu(relu_s[B], h_slice, agg_ps[:], b_ap)
                    if li < 2:
                        return
                    for c, (pg0, pg1) in enumerate(POOL_CH):
                        if g0 < pg1 <= g1:
                            emit_pool(c)
                    if g1 == 40:
                        # drug[0:130] ready (pool chunk 0 fired at B=3, one
                        # batch of slack so the PE queue doesn't wait on it)
                        emit_mlp(0)
                        nc.sync.dma_start(
                            out=OUT[:, 0:MLP_CH[0][1]],
                            in_=out_sb[:, 0:MLP_CH[0][1]])
                    if g1 == GROUPS:
                        emit_mlp(1)

                # layer 1's hW stream depends only on xT (resident early),
                # so run it fully ahead of the AT-gated aggs
                skew = N_BATCH if li == 0 else 3
                pend = []
                hw_tiles = {}
                for b in range(N_BATCH):
                    hw_ps = mmpool.tile([HID, BATCH * HID], F32, tag="mm")
                    for gi, g in enumerate(range(b * BATCH, (b + 1) * BATCH)):
                        nc.tensor.matmul(
                            out=hw_ps[:, gi * HID:(gi + 1) * HID],
                            lhsT=h_in[:, g * GS:g * GS + GS][0:kdim, :],
                            rhs=w_ap,
                            start=True, stop=True,
                        )
                    hw_tiles[b] = hwpool.tile([HID, BATCH * HID], F16,
                                              name="hw_sb")
                    emit_cast(cast_s[b], hw_tiles[b][:], hw_ps[:])
                    B = (b - skew) // 2
                    if b - skew >= 0 and (b - skew) % 2 == 1:
                        emit_agg(B, (hw_tiles.pop(2 * B),
                                     hw_tiles.pop(2 * B + 1)))
                for B in range((max(0, N_BATCH - skew) + 1) // 2, N_ABATCH):
                    tiles = [hw_tiles.pop(k) for k in
                             sorted(hw_tiles) if 2 * B <= k < 2 * B + 2]
                    emit_agg(B, tiles)

            nc.sync.dma_start(out=OUT[:, MLP_CH[0][1]:GPC],
                              in_=out_sb[:, MLP_CH[0][1]:GPC])

    _split_multi_waits(nc)
    return nc


_NC = None


def _get_program():
    global _NC
    if _NC is None:
        _NC = _build_program()
    return _NC


def _prep_inputs(x, edge_index, batch, prot_vec,
                 W1, b1, W2, b2, W3, b3, Wf1, bf1, Wf2, bf2):
    x = np.ascontiguousarray(np.asarray(x, np.float32))
    src = np.asarray(edge_index[0], np.int64)
    dst = np.asarray(edge_index[1], np.int64)

    assert (src // NPG == dst // NPG).all(), "edges must stay within graphs"
    deg = np.bincount(dst, minlength=N_NODES).astype(np.float32) + 1.0
    dinv = (1.0 / np.sqrt(deg)).astype(np.float32)
    coef = (dinv[src] * dinv[dst]).astype(np.float64)

    # AT[g, u, v] = sum of dinv[su]*dinv[sv] over edges (u -> v) + diag dinv^2
    flat = (src * NPG + dst % NPG).astype(np.int64)
    A = np.bincount(flat, weights=coef, minlength=N_NODES * NPG)
    A = A.astype(np.float32).reshape(N_GRAPHS, NPG, NPG)
    di = np.arange(NPG)
    A[:, di, di] += (dinv * dinv).reshape(N_GRAPHS, NPG)

    # per-core block-diagonal layout [GW, COLS_A], fp8e4m3
    A_pad = np.zeros((N_CORES, PAD_G, NPG, NPG), np.float32)
    A_pad[:, :GPC] = A.reshape(N_CORES, GPC, NPG, NPG)
    AT_full = np.zeros((N_CORES, GW, GROUPS, GPG, NPG), np.float32)
    Ar = A_pad.reshape(N_CORES, GROUPS, GPG, NPG, NPG)
    for j in range(GPG):
        AT_full[:, NPG * j:NPG * (j + 1), :, j, :] = \
            Ar[:, :, j].transpose(0, 2, 1, 3)
    AT_full = np.ascontiguousarray(
        AT_full.reshape(N_CORES, GW, COLS_A).astype(ml_dtypes.float8_e4m3))

    # xT with the 128-wide group stride of the H layout (fp16: fp8 as a
    # matmul STATIONARY operand wedged the device; only the moving AT
    # operand is fp8)
    xm = x.reshape(N_CORES, GPC * NPG, F_IN).transpose(0, 2, 1)  # [c,13,6400]
    xT = np.zeros((N_CORES, F_IN, GROUPS, GS), np.float16)
    full = (GPC * NPG) // GW       # 51 full groups
    xT[:, :, :full, :GW] = xm[:, :, :full * GW].reshape(
        N_CORES, F_IN, full, GW)
    rem = GPC * NPG - full * GW    # 25 leftover cols (graph 255)
    if rem:
        xT[:, :, full, :rem] = xm[:, :, full * GW:]
    xT = np.ascontiguousarray(xT.reshape(N_CORES, F_IN, COLS_H))

    # packed fp16 const blob [128, CW_COLS] per core (PT differs per core)
    cw = np.zeros((N_CORES, HID, CW_COLS), np.float16)
    w1 = np.asarray(W1, np.float16)
    cw[:, :, CW_W2:CW_W2 + HID] = np.asarray(W2, np.float16)
    cw[:, :, CW_W3:CW_W3 + HID] = np.asarray(W3, np.float16)
    wf1 = np.asarray(Wf1, np.float16)
    cw[:, :, CW_WF1:CW_WF1 + 256] = wf1[0:HID]
    cw[:, :, CW_WF1 + 256:CW_WF1 + 512] = wf1[HID:2 * HID]
    wf2 = np.asarray(Wf2, np.float16)
    cw[:, :, CW_WF2] = wf2[0:HID, 0]
    cw[:, :, CW_WF2 + 1] = wf2[HID:256, 0]
    cw[:, :, CW_PT:CW_PT + GPC] = (
        np.asarray(prot_vec, np.float16).reshape(N_CORES, GPC, PROT)
        .transpose(0, 2, 1))
    cw = np.ascontiguousarray(cw)

    cb = np.zeros((HID, 6), np.float32)
    cb[:, 0] = np.asarray(b1, np.float32)
    cb[:, 1] = np.asarray(b2, np.float32)
    cb[:, 2] = np.asarray(b3, np.float32)
    bf1_ = np.asarray(bf1, np.float32)
    cb[:, 3] = bf1_[0:HID]
    cb[:, 4] = bf1_[HID:256]
    cb[0, 5] = np.asarray(bf2, np.float32).reshape(-1)[0]

    in_maps = []
    for c in range(N_CORES):
        in_maps.append({
            "xT": xT[c], "AT": AT_full[c], "W1B": w1, "CW": cw[c], "CB": cb,
        })
    return in_maps


def _run(inputs, **run_kwargs):
    in_maps = _prep_inputs(**inputs)
    nc = _get_program()
    res = run_bass_kernel_spmd(nc, in_maps, core_ids=list(range(N_CORES)),
                               **run_kwargs)
    out = np.concatenate(
        [r["out"].reshape(GPC, 1) for r in res.results], axis=0)
    return out.astype(np.float32), res


def kernel(**inputs):
    out, _ = _run(inputs)
    return out
